# revision 11
# baseline (speedup 1.0000x reference)
"""Trainium2 Bass kernel: masked multi-head attention with doc-aware RoPE.

Problem shape: x[2, 2048, 2048], 16 heads x head_dim 128, doc-blockwise causal
mask with positions reset per document, out-proj with bias.

Sharding over 8 NeuronCores: core c = 4*b + g handles batch b (2) and head
group g (4 heads, i.e. 512 of the 2048 feature columns of Wq/Wk/Wv and 512
rows of Wo). Each core computes an out-proj partial [2048, 2048]; the host
sums the 4 partials per batch and adds the bias.

Device program (identical structure on all 8 cores; compiled per input batch
because the doc-boundary structure specializes the attention loop bounds):
  Phase 1: Q^T/K^T (layout [head_dim, S]) and V (layout [S, head_dim]) via
           matmuls against x^T; doc-aware RoPE applied to Q/K with a
           pair-rotation matmul (PE) + cos/sin elementwise combine (DVE).
  Phase 2: attention per (head, 512-query block): S^T tiles [128 keys, 512
           queries] on PE, multiplicative {0,1} doc-causal mask (host-built,
           streamed), exp on ACT (no max-subtraction: scores ~ N(0,1)),
           row-sums via all-ones matmul (lands replicated across partitions),
           P^T V accumulated into ctx^T in PSUM, normalized by reciprocal.
           Only key tiles in [doc_start(query block), query block end) are
           touched (doc-block sparsity).
  Phase 3: out partial[s, e] += ctx^T_h[t, s] @ Wo_h[t, e] accumulated over
           the 4 local heads in PSUM, streamed to DRAM.

All matmuls run in bf16 (1 cycle/row on the PE; fp32 would be 4x slower).
"""

import json

import numpy as np
import ml_dtypes

B, S, D, H = 2, 2048, 2048, 16
HD = D // H  # 128
ROPE_BASE = 10000.0
G = 4            # head groups (cores per batch)
HPG = H // G     # heads per group = 4
GC = HPG * HD    # feature cols per group = 512
N_CORES = 8
IB = 512         # query block size (phase 2)
N_IB = S // IB   # 4
JT = 128         # key tile size
BF = None        # set lazily (mybir.dt.bfloat16)
F32 = None

_DMA_OPS = {"DMACopy", "DMATranspose", "CCE", "DMAListExec"}


def _legalize_bir(raw: bytes) -> bytes:
    """This container's walrus accepts at most one sem-wait and one sem-update
    per instruction; Tile freely attaches several. Hoist extra waits onto
    NoOps inserted before the instruction (same engine stream -> the
    sequencer stalls there first, which is equivalent but legal) and extra
    updates onto NoOps after it (engine queues are FIFO, so the trailing nop's
    update fires after the instruction completes). DMA updates are left
    untouched: they ride the DGE descriptor, not the instruction."""
    d = json.loads(raw)
    n = 0
    for fn in d["functions"]:
        for blk in fn["blocks"]:
            out = []
            for inst in blk["instructions"]:
                si = inst.get("sync_info")
                if not si:
                    out.append(inst)
                    continue
                waits = list(si.get("on_wait") or [])
                upds = list(si.get("on_update") or [])
                dbg = {"debug": inst["debug"]} if "debug" in inst else {}
                while len(waits) > 1:
                    w = waits.pop(0)
                    n += 1
                    out.append({
                        "name": f"lglw-{n}", "opcode": "NoOp",
                        "engine": inst["engine"], "ins": [], "outs": [],
                        "sync_info": {"on_wait": [w], "on_update": []}, **dbg,
                    })
                si["on_wait"] = waits
                trailing = []
                if inst["opcode"] not in _DMA_OPS:
                    while len(upds) > 1:
                        u = upds.pop()
                        n += 1
                        trailing.append({
                            "name": f"lglu-{n}", "opcode": "NoOp",
                            "engine": inst["engine"], "ins": [], "outs": [],
                            "sync_info": {"on_wait": [], "on_update": [u]}, **dbg,
                        })
                    si["on_update"] = upds
                out.append(inst)
                out.extend(trailing)
            blk["instructions"] = out
    return json.dumps(d).encode()


def _doc_position_ids(doc_row: np.ndarray) -> np.ndarray:
    s = doc_row.shape[0]
    ar = np.arange(s)
    boundary = np.ones(s, dtype=bool)
    boundary[1:] = doc_row[1:] != doc_row[:-1]
    last = np.maximum.accumulate(np.where(boundary, ar, 0))
    return ar - last


def _doc_starts(doc_row: np.ndarray) -> np.ndarray:
    s = doc_row.shape[0]
    ar = np.arange(s)
    boundary = np.ones(s, dtype=bool)
    boundary[1:] = doc_row[1:] != doc_row[:-1]
    return np.maximum.accumulate(np.where(boundary, ar, 0))


def _rope_tables(doc_row: np.ndarray):
    pos = _doc_position_ids(doc_row).astype(np.float32)          # [S]
    inv_freq = 1.0 / (ROPE_BASE ** (np.arange(0, HD, 2, dtype=np.float32) / HD))
    freqs = pos[:, None] * inv_freq[None, :]                     # [S, 64]
    cos = np.repeat(np.cos(freqs), 2, axis=-1)                   # [S, 128]
    sin = np.repeat(np.sin(freqs), 2, axis=-1)
    to_bf = lambda a: np.ascontiguousarray(a.T).astype(ml_dtypes.bfloat16)
    return to_bf(cos), to_bf(sin)                                # [128, S]


def _build_program(jts_per_ib):
    import concourse.bass as bass
    import concourse.mybir as mybir
    from concourse.tile import TileContext

    BF = mybir.dt.bfloat16
    F32 = mybir.dt.float32
    Exp = mybir.ActivationFunctionType.Exp
    Ln = mybir.ActivationFunctionType.Ln
    NDT = D // 128  # 16 contraction tiles
    n_units = sum(len(j) for j in jts_per_ib)
    max_jts = max(len(j) for j in jts_per_ib)

    nc = bass.Bass("TRN2", num_devices=N_CORES)
    xt_d = nc.dram_tensor("xt", [D, S], BF, kind="ExternalInput")
    wq_d = nc.dram_tensor("wq", [D, GC], BF, kind="ExternalInput")
    wk_d = nc.dram_tensor("wk", [D, GC], BF, kind="ExternalInput")
    wv_d = nc.dram_tensor("wv", [D, GC], BF, kind="ExternalInput")
    wo_d = nc.dram_tensor("wo", [GC, D], BF, kind="ExternalInput")
    cos_d = nc.dram_tensor("cosT", [HD, S], BF, kind="ExternalInput")
    sin_d = nc.dram_tensor("sinT", [HD, S], BF, kind="ExternalInput")
    rot_d = nc.dram_tensor("rotm", [HD, HD], BF, kind="ExternalInput")
    idn_d = nc.dram_tensor("idn", [128, 128], BF, kind="ExternalInput")
    msk_d = nc.dram_tensor("masks", [max(n_units, 1), JT, IB], BF, kind="ExternalInput")
    out_d = nc.dram_tensor("part", [S, D], F32, kind="ExternalOutput")

    with TileContext(nc) as tc:
        with (
            tc.tile_pool(name="const", bufs=1) as constp,
            tc.tile_pool(name="persist", bufs=1) as persist,
            tc.tile_pool(name="xt", bufs=18) as xtp,
            tc.tile_pool(name="w2", bufs=2) as w2,
            tc.tile_pool(name="w3", bufs=3) as w3,
            tc.tile_pool(name="mskp", bufs=12) as mskp,
            tc.tile_pool(name="ps_mm", bufs=3, space="PSUM") as ps_mm,
            tc.tile_pool(name="ps_rot", bufs=2, space="PSUM") as ps_rot,
            tc.tile_pool(name="ps_ctx", bufs=2, space="PSUM") as ps_ctx,
            tc.tile_pool(name="ps_l", bufs=1, space="PSUM") as ps_l,
        ):
            # ---- constants / weights resident in SBUF
            wq = constp.tile([128, NDT, GC], BF, tag="wq")
            wk = constp.tile([128, NDT, GC], BF, tag="wk")
            wv = constp.tile([128, NDT, GC], BF, tag="wv")
            wo = constp.tile([128, HPG, D], BF, tag="wo")
            cosT = constp.tile([HD, S], BF, tag="cos")
            sinT = constp.tile([HD, S], BF, tag="sin")
            rotm = constp.tile([HD, HD], BF, tag="rot")
            idn = constp.tile([128, 128], BF, tag="idn")
            ones = constp.tile([128, 128], BF, tag="ones")
            xt_view = xt_d.ap().rearrange("(dt p) s -> p dt s", p=128)
            xt_first = []
            for dt in range(NDT):
                t = xtp.tile([128, IB], BF, tag="xt", name=f"xt0_{dt}")
                nc.sync.dma_start(out=t, in_=xt_view[:, dt, 0:IB])
                xt_first.append(t)
            for w_t, w_dram in ((wv, wv_d), (wq, wq_d), (wk, wk_d)):
                nc.sync.dma_start(
                    out=w_t, in_=w_dram.ap().rearrange("(dt p) c -> p dt c", p=128))
            nc.sync.dma_start(out=cosT, in_=cos_d[:, :])
            nc.sync.dma_start(out=sinT, in_=sin_d[:, :])
            nc.sync.dma_start(out=rotm, in_=rot_d[:, :])
            nc.sync.dma_start(out=idn, in_=idn_d[:, :])
            nc.vector.memset(ones, 1.0)

            # ---- persistent activations
            qT = [persist.tile([HD, S], BF, tag=f"q{h}", name=f"qT{h}") for h in range(HPG)]
            kT = [persist.tile([HD, S], BF, tag=f"k{h}", name=f"kT{h}") for h in range(HPG)]
            v_sb = persist.tile([128, NDT, GC], BF, tag="v")
            ctxT = [persist.tile([HD, S], BF, tag=f"c{h}", name=f"ctxT{h}") for h in range(HPG)]

            # ================= Phase 1: projections + RoPE =================
            for sb in range(N_IB):
                scol = slice(sb * IB, (sb + 1) * IB)
                if sb == 0:
                    xt_t = xt_first
                else:
                    xt_t = []
                    for dt in range(NDT):
                        t = xtp.tile([128, IB], BF, tag="xt", name=f"xt{sb}_{dt}")
                        nc.sync.dma_start(out=t, in_=xt_view[:, dt, scol])
                        xt_t.append(t)

                # V for the 4 s-tiles of this block: out[s 128, t 512]
                for sl in range(IB // 128):
                    st = sb * 4 + sl
                    ps = ps_mm.tile([128, GC], F32, tag="mm")
                    for dt in range(NDT):
                        nc.tensor.matmul(
                            ps, xt_t[dt][:, sl * 128:(sl + 1) * 128],
                            wv[:, dt, :], start=(dt == 0), stop=(dt == NDT - 1))
                    nc.scalar.copy(v_sb[:, st, :], ps)

                # Q^T / K^T per head with RoPE: out[t 128, s 512]
                for w_t, dsts in ((wq, qT), (wk, kT)):
                    for h in range(HPG):
                        hcol = slice(h * HD, (h + 1) * HD)
                        ps = ps_mm.tile([128, IB], F32, tag="mm")
                        for dt in range(NDT):
                            nc.tensor.matmul(
                                ps, w_t[:, dt, hcol], xt_t[dt],
                                start=(dt == 0), stop=(dt == NDT - 1))
                        qtmp = w2.tile([128, IB], BF, tag="sc")
                        nc.scalar.copy(qtmp, ps)
                        rps = ps_rot.tile([128, IB], F32, tag="rot")
                        nc.tensor.matmul(rps, rotm, qtmp, start=True, stop=True)
                        # RoPE combine: q*cos + rot(q)*sin
                        t1 = w2.tile([128, IB], F32, tag="a")
                        nc.vector.tensor_mul(t1, ps, cosT[:, scol])
                        t2 = w2.tile([128, IB], F32, tag="b")
                        nc.vector.tensor_mul(t2, rps, sinT[:, scol])
                        nc.vector.tensor_add(dsts[h][:, scol], t1, t2)

            # ============ Phase 2+3 interleaved per query block ============
            nc.sync.dma_start(
                out=wo, in_=wo_d.ap().rearrange("(h p) e -> p h e", p=128))
            unit_off = [0]
            for jts in jts_per_ib:
                unit_off.append(unit_off[-1] + len(jts))
            all_mts = []
            for ib in range(N_IB):
                mts = []
                for idx, jt in enumerate(jts_per_ib[ib]):
                    mt = mskp.tile([JT, IB], BF, tag="msk", name=f"m{ib}_{jt}")
                    nc.sync.dma_start(out=mt, in_=msk_d[unit_off[ib] + idx])
                    mts.append(mt)
                all_mts.append(mts)
            for ib in range(N_IB):
                icol = slice(ib * IB, (ib + 1) * IB)
                jts = jts_per_ib[ib]
                mts = all_mts[ib]
                for h in range(HPG):
                    hcol = slice(h * HD, (h + 1) * HD)
                    ctx_ps = ps_ctx.tile([128, IB], F32, tag="ctx")
                    l_ps = ps_l.tile([128, IB], F32, tag="l")
                    for idx, jt in enumerate(jts):
                        first, last = idx == 0, idx == len(jts) - 1
                        jcol = slice(jt * JT, (jt + 1) * JT)
                        st_ps = ps_mm.tile([128, IB], F32, tag="mm")
                        nc.tensor.matmul(st_ps, kT[h][:, jcol], qT[h][:, icol],
                                         start=True, stop=False)
                        # additive doc-causal mask: st += I.T @ M  (M is 0/-1e9)
                        nc.tensor.matmul(st_ps, idn, mts[idx],
                                         start=False, stop=True)
                        pt = w3.tile([128, IB], BF, tag="exp")
                        nc.scalar.activation(pt, st_ps, Exp)
                        nc.tensor.matmul(ctx_ps, v_sb[:, jt, hcol], pt,
                                         start=first, stop=last)
                        nc.tensor.matmul(l_ps, ones, pt, start=first, stop=last)
                    # evacuate l/ctx banks fast (a slow reader here would
                    # stall the next accumulation group at the PE queue head)
                    l_sb = w2.tile([128, IB], F32, tag="a")
                    nc.vector.tensor_copy(l_sb, l_ps)
                    ctx_sb = w2.tile([128, IB], F32, tag="b")
                    nc.vector.tensor_copy(ctx_sb, ctx_ps)
                    rec = w2.tile([128, IB], F32, tag="rec")
                    nc.vector.reciprocal(rec, l_sb)
                    nc.vector.tensor_mul(ctxT[h][:, icol], ctx_sb, rec)
                # out rows for this block: out[s, e] = sum_h ctxT_h.T @ Wo_h
                for eb in range(D // 512):
                    ecol = slice(eb * 512, (eb + 1) * 512)
                    for sl in range(IB // 128):
                        srow = slice(ib * IB + sl * 128, ib * IB + (sl + 1) * 128)
                        ops = ps_rot.tile([128, 512], F32, tag="rot")
                        for h in range(HPG):
                            nc.tensor.matmul(ops, ctxT[h][:, srow], wo[:, h, ecol],
                                             start=(h == 0), stop=(h == HPG - 1))
                        osb = w2.tile([128, 512], F32, tag="sc")
                        if (eb + sl) % 2 == 0:
                            nc.vector.tensor_copy(osb, ops)
                        else:
                            nc.scalar.copy(osb, ops)
                        nc.sync.dma_start(out=out_d[srow, ecol], in_=osb)

    orig = nc.to_json_bytes
    nc.to_json_bytes = lambda: _legalize_bir(orig())
    return nc


LAST_RUN = {}


def kernel(x, doc_ids, Wq, Wk, Wv, Wo, bo):
    import os
    from concourse.bass_utils import run_bass_kernel_spmd

    x = np.asarray(x, dtype=np.float32)
    doc_ids = np.asarray(doc_ids)
    Wq = np.asarray(Wq, dtype=np.float32)
    Wk = np.asarray(Wk, dtype=np.float32)
    Wv = np.asarray(Wv, dtype=np.float32)
    Wo = np.asarray(Wo, dtype=np.float32)
    bo = np.asarray(bo, dtype=np.float32)

    bf = ml_dtypes.bfloat16
    inv_scale = 1.0 / np.sqrt(HD)

    # doc structure -> shared (batch-unified) attention spans
    starts = [_doc_starts(doc_ids[b]) for b in range(B)]
    jts_per_ib = []
    for ib in range(N_IB):
        ja = min(int(starts[b][ib * IB]) for b in range(B))
        jts_per_ib.append(list(range(ja // JT, (ib + 1) * IB // JT)))

    # masks per batch: multiplicative {0,1}, doc-equality AND causal
    masks = []
    for b in range(B):
        drow = doc_ids[b]
        blocks = []
        for ib in range(N_IB):
            i0 = ib * IB
            for jt in jts_per_ib[ib]:
                j0 = jt * JT
                mj = drow[j0:j0 + JT, None] == drow[None, i0:i0 + IB]
                cz = (np.arange(j0, j0 + JT)[:, None]
                      <= np.arange(i0, i0 + IB)[None, :])
                allowed = mj & cz
                blocks.append(np.where(allowed, 0.0, -1e9).astype(bf))
        masks.append(np.stack(blocks) if blocks
                     else np.zeros((1, JT, IB), dtype=bf))

    rope = [_rope_tables(doc_ids[b]) for b in range(B)]
    rotm = np.zeros((HD, HD), dtype=bf)
    for t in range(HD // 2):
        rotm[2 * t + 1, 2 * t] = -1.0
        rotm[2 * t, 2 * t + 1] = 1.0

    xts = [np.ascontiguousarray(x[b].T).astype(bf) for b in range(B)]
    wq_s = (Wq * inv_scale).astype(bf)
    wk_s = Wk.astype(bf)
    wv_s = Wv.astype(bf)
    wo_s = Wo.astype(bf)

    in_maps = []
    for c in range(N_CORES):
        b, g = divmod(c, G)
        cols = slice(g * GC, (g + 1) * GC)
        in_maps.append({
            "xt": xts[b],
            "wq": np.ascontiguousarray(wq_s[:, cols]),
            "wk": np.ascontiguousarray(wk_s[:, cols]),
            "wv": np.ascontiguousarray(wv_s[:, cols]),
            "wo": np.ascontiguousarray(wo_s[cols, :]),
            "cosT": rope[b][0],
            "sinT": rope[b][1],
            "rotm": rotm,
            "idn": np.eye(128, dtype=bf),
            "masks": masks[b],
        })

    nc = _build_program(jts_per_ib)
    trace = bool(int(os.environ.get("KERNEL_TRACE", "0")))
    res = run_bass_kernel_spmd(nc, in_maps, core_ids=list(range(N_CORES)),
                               trace=trace)
    LAST_RUN["exec_time_ns"] = res.exec_time_ns
    LAST_RUN["mean_exec_time_ns"] = res.mean_exec_time_ns
    LAST_RUN["trace"] = res.instructions_and_trace

    out = np.empty((B, S, D), dtype=np.float32)
    for b in range(B):
        acc = np.zeros((S, D), dtype=np.float32)
        for g in range(G):
            acc += res.results[b * G + g]["part"]
        out[b] = acc + bo[None, :]
    return out


# revision 12
# speedup vs baseline: 1.1767x; 1.1767x over previous
"""Trainium2 Bass kernel: masked multi-head attention with doc-aware RoPE.

Problem shape: x[2, 2048, 2048], 16 heads x head_dim 128, doc-blockwise causal
mask with positions reset per document, out-proj with bias.

Sharding over 8 NeuronCores: core c = 4*b + g handles batch b (2) and head
group g (4 heads, i.e. 512 of the 2048 feature columns of Wq/Wk/Wv and 512
rows of Wo). Each core computes an out-proj partial [2048, 2048]; the host
sums the 4 partials per batch and adds the bias.

Device program (identical structure on all 8 cores; compiled per input batch
because the doc-boundary structure specializes the attention loop bounds):
  Phase 1: Q^T/K^T (layout [head_dim, S]) and V (layout [S, head_dim]) via
           matmuls against x^T; doc-aware RoPE applied to Q/K with a
           pair-rotation matmul (PE) + cos/sin elementwise combine (DVE).
  Phase 2: attention per (head, 512-query block): S^T tiles [128 keys, 512
           queries] on PE, multiplicative {0,1} doc-causal mask (host-built,
           streamed), exp on ACT (no max-subtraction: scores ~ N(0,1)),
           row-sums via all-ones matmul (lands replicated across partitions),
           P^T V accumulated into ctx^T in PSUM, normalized by reciprocal.
           Only key tiles in [doc_start(query block), query block end) are
           touched (doc-block sparsity).
  Phase 3: out partial[s, e] += ctx^T_h[t, s] @ Wo_h[t, e] accumulated over
           the 4 local heads in PSUM, streamed to DRAM.

All matmuls run in bf16 (1 cycle/row on the PE; fp32 would be 4x slower).
"""

import json

import numpy as np
import ml_dtypes

B, S, D, H = 2, 2048, 2048, 16
HD = D // H  # 128
ROPE_BASE = 10000.0
G = 4            # head groups (cores per batch)
HPG = H // G     # heads per group = 4
GC = HPG * HD    # feature cols per group = 512
N_CORES = 8
IB = 512         # query block size (phase 2)
N_IB = S // IB   # 4
JT = 128         # key tile size
BF = None        # set lazily (mybir.dt.bfloat16)
F32 = None

_DMA_OPS = {"DMACopy", "DMATranspose", "CCE", "DMAListExec"}


def _legalize_bir(raw: bytes) -> bytes:
    """This container's walrus accepts at most one sem-wait and one sem-update
    per instruction; Tile freely attaches several. Hoist extra waits onto
    NoOps inserted before the instruction (same engine stream -> the
    sequencer stalls there first, which is equivalent but legal) and extra
    updates onto NoOps after it (engine queues are FIFO, so the trailing nop's
    update fires after the instruction completes). DMA updates are left
    untouched: they ride the DGE descriptor, not the instruction."""
    d = json.loads(raw)
    n = 0
    for fn in d["functions"]:
        for blk in fn["blocks"]:
            out = []
            for inst in blk["instructions"]:
                si = inst.get("sync_info")
                if not si:
                    out.append(inst)
                    continue
                waits = list(si.get("on_wait") or [])
                upds = list(si.get("on_update") or [])
                dbg = {"debug": inst["debug"]} if "debug" in inst else {}
                while len(waits) > 1:
                    w = waits.pop(0)
                    n += 1
                    out.append({
                        "name": f"lglw-{n}", "opcode": "NoOp",
                        "engine": inst["engine"], "ins": [], "outs": [],
                        "sync_info": {"on_wait": [w], "on_update": []}, **dbg,
                    })
                si["on_wait"] = waits
                trailing = []
                if inst["opcode"] not in _DMA_OPS:
                    while len(upds) > 1:
                        u = upds.pop()
                        n += 1
                        trailing.append({
                            "name": f"lglu-{n}", "opcode": "NoOp",
                            "engine": inst["engine"], "ins": [], "outs": [],
                            "sync_info": {"on_wait": [], "on_update": [u]}, **dbg,
                        })
                    si["on_update"] = upds
                out.append(inst)
                out.extend(trailing)
            blk["instructions"] = out
    return json.dumps(d).encode()


def _doc_position_ids(doc_row: np.ndarray) -> np.ndarray:
    s = doc_row.shape[0]
    ar = np.arange(s)
    boundary = np.ones(s, dtype=bool)
    boundary[1:] = doc_row[1:] != doc_row[:-1]
    last = np.maximum.accumulate(np.where(boundary, ar, 0))
    return ar - last


def _doc_starts(doc_row: np.ndarray) -> np.ndarray:
    s = doc_row.shape[0]
    ar = np.arange(s)
    boundary = np.ones(s, dtype=bool)
    boundary[1:] = doc_row[1:] != doc_row[:-1]
    return np.maximum.accumulate(np.where(boundary, ar, 0))


def _rope_tables(doc_row: np.ndarray):
    pos = _doc_position_ids(doc_row).astype(np.float32)          # [S]
    inv_freq = 1.0 / (ROPE_BASE ** (np.arange(0, HD, 2, dtype=np.float32) / HD))
    freqs = pos[:, None] * inv_freq[None, :]                     # [S, 64]
    cos = np.repeat(np.cos(freqs), 2, axis=-1)                   # [S, 128]
    sin = np.repeat(np.sin(freqs), 2, axis=-1)
    to_bf = lambda a: np.ascontiguousarray(a.T).astype(ml_dtypes.bfloat16)
    return to_bf(cos), to_bf(sin)                                # [128, S]


def _build_program(jts_per_ib):
    import concourse.bass as bass
    import concourse.mybir as mybir
    from concourse.tile import TileContext

    BF = mybir.dt.bfloat16
    F32 = mybir.dt.float32
    Exp = mybir.ActivationFunctionType.Exp
    NDT = D // 128  # 16 contraction tiles
    n_units = sum(len(j) for j in jts_per_ib)

    nc = bass.Bass("TRN2", num_devices=N_CORES)
    xt_d = nc.dram_tensor("xt", [D, S], BF, kind="ExternalInput")
    wq_d = nc.dram_tensor("wq", [D, GC], BF, kind="ExternalInput")
    wk_d = nc.dram_tensor("wk", [D, GC], BF, kind="ExternalInput")
    wv_d = nc.dram_tensor("wv", [D, GC], BF, kind="ExternalInput")
    wo_d = nc.dram_tensor("wo", [GC, D], BF, kind="ExternalInput")
    cos_d = nc.dram_tensor("cosT", [HD, S], BF, kind="ExternalInput")
    sin_d = nc.dram_tensor("sinT", [HD, S], BF, kind="ExternalInput")
    rot_d = nc.dram_tensor("rotm", [HD, HD], BF, kind="ExternalInput")
    idn_d = nc.dram_tensor("idn", [128, 128], BF, kind="ExternalInput")
    msk_d = nc.dram_tensor("masks", [max(n_units, 1), JT, IB], BF, kind="ExternalInput")
    out_d = nc.dram_tensor("part", [S, D], F32, kind="ExternalOutput")

    with TileContext(nc) as tc:
        with (
            tc.tile_pool(name="constA", bufs=1) as constA,
            tc.tile_pool(name="persist", bufs=1) as persist,
            tc.tile_pool(name="ps_mm", bufs=3, space="PSUM") as ps_mm,
            tc.tile_pool(name="ps_rot", bufs=2, space="PSUM") as ps_rot,
            tc.tile_pool(name="ps_ctx", bufs=2, space="PSUM") as ps_ctx,
            tc.tile_pool(name="ps_l", bufs=1, space="PSUM") as ps_l,
        ):
            # phase-2/3 constants (DMAs for these are emitted after phase 1)
            wo = constA.tile([128, HPG, D], BF, tag="wo")
            idn = constA.tile([128, 128], BF, tag="idn")
            ones = constA.tile([128, 128], BF, tag="ones")
            qT = [persist.tile([HD, S], BF, tag=f"q{h}", name=f"qT{h}") for h in range(HPG)]
            kT = [persist.tile([HD, S], BF, tag=f"k{h}", name=f"kT{h}") for h in range(HPG)]
            v_sb = persist.tile([128, NDT, GC], BF, tag="v")
            ctxT = [persist.tile([HD, S], BF, tag=f"c{h}", name=f"ctxT{h}") for h in range(HPG)]

            xt_view = xt_d.ap().rearrange("(dt p) s -> p dt s", p=128)

            # ============ Phase 1 (pools freed afterwards) ============
            with (
                tc.tile_pool(name="p1c", bufs=1) as p1c,
                tc.tile_pool(name="p1xt", bufs=34) as p1xt,
                tc.tile_pool(name="p1w", bufs=2) as p1w,
            ):
                wq = p1c.tile([128, NDT, GC], BF, tag="wq")
                wk = p1c.tile([128, NDT, GC], BF, tag="wk")
                wv = p1c.tile([128, NDT, GC], BF, tag="wv")
                cosT = p1c.tile([HD, S], BF, tag="cos")
                sinT = p1c.tile([HD, S], BF, tag="sin")
                rotm = p1c.tile([HD, HD], BF, tag="rot")
                # first x block + V weights first: compute starts ~15us in
                xt_first = []
                for dt in range(NDT):
                    t = p1xt.tile([128, IB], BF, tag="xt", name=f"xt0_{dt}")
                    nc.sync.dma_start(out=t, in_=xt_view[:, dt, 0:IB])
                    xt_first.append(t)
                for w_t, w_dram in ((wv, wv_d), (wq, wq_d), (wk, wk_d)):
                    nc.sync.dma_start(
                        out=w_t,
                        in_=w_dram.ap().rearrange("(dt p) c -> p dt c", p=128))
                nc.sync.dma_start(out=cosT, in_=cos_d[:, :])
                nc.sync.dma_start(out=sinT, in_=sin_d[:, :])
                nc.sync.dma_start(out=rotm, in_=rot_d[:, :])
                nc.sync.dma_start(out=idn, in_=idn_d[:, :])
                nc.vector.memset(ones, 1.0)

                for sb in range(N_IB):
                    scol = slice(sb * IB, (sb + 1) * IB)
                    if sb == 0:
                        xt_t = xt_first
                    else:
                        xt_t = []
                        for dt in range(NDT):
                            t = p1xt.tile([128, IB], BF, tag="xt",
                                          name=f"xt{sb}_{dt}")
                            nc.sync.dma_start(out=t, in_=xt_view[:, dt, scol])
                            xt_t.append(t)

                    # V for the 4 s-tiles of this block: out[s 128, t 512]
                    for sl in range(IB // 128):
                        st = sb * 4 + sl
                        ps = ps_mm.tile([128, GC], F32, tag="mm")
                        for dt in range(NDT):
                            nc.tensor.matmul(
                                ps, xt_t[dt][:, sl * 128:(sl + 1) * 128],
                                wv[:, dt, :], start=(dt == 0), stop=(dt == NDT - 1))
                        nc.scalar.copy(v_sb[:, st, :], ps)

                    # Q^T / K^T per head with RoPE: out[t 128, s 512]
                    for w_t, dsts in ((wq, qT), (wk, kT)):
                        for h in range(HPG):
                            hcol = slice(h * HD, (h + 1) * HD)
                            ps = ps_mm.tile([128, IB], F32, tag="mm")
                            for dt in range(NDT):
                                nc.tensor.matmul(
                                    ps, w_t[:, dt, hcol], xt_t[dt],
                                    start=(dt == 0), stop=(dt == NDT - 1))
                            qtmp = p1w.tile([128, IB], BF, tag="qtmp")
                            nc.scalar.copy(qtmp, ps)
                            rps = ps_rot.tile([128, IB], F32, tag="rot")
                            nc.tensor.matmul(rps, rotm, qtmp, start=True, stop=True)
                            # RoPE combine: q*cos + rot(q)*sin
                            t1 = p1w.tile([128, IB], F32, tag="t1")
                            nc.vector.tensor_mul(t1, ps, cosT[:, scol])
                            t2 = p1w.tile([128, IB], F32, tag="t2")
                            nc.vector.tensor_mul(t2, rps, sinT[:, scol])
                            nc.vector.tensor_add(dsts[h][:, scol], t1, t2)

            # ============ Phase 2+3 interleaved per query block ============
            with (
                tc.tile_pool(name="mskp", bufs=n_units + 1) as mskp,
                tc.tile_pool(name="w2", bufs=2) as w2,
                tc.tile_pool(name="w3", bufs=3) as w3,
            ):
                nc.sync.dma_start(
                    out=wo, in_=wo_d.ap().rearrange("(h p) e -> p h e", p=128))
                unit_off = [0]
                for jts in jts_per_ib:
                    unit_off.append(unit_off[-1] + len(jts))
                all_mts = []
                for ib in range(N_IB):
                    mts = []
                    for idx, jt in enumerate(jts_per_ib[ib]):
                        mt = mskp.tile([JT, IB], BF, tag="msk", name=f"m{ib}_{jt}")
                        nc.sync.dma_start(out=mt, in_=msk_d[unit_off[ib] + idx])
                        mts.append(mt)
                    all_mts.append(mts)

                for ib in range(N_IB):
                    icol = slice(ib * IB, (ib + 1) * IB)
                    jts = jts_per_ib[ib]
                    mts = all_mts[ib]
                    for h in range(HPG):
                        hcol = slice(h * HD, (h + 1) * HD)
                        ctx_ps = ps_ctx.tile([128, IB], F32, tag="ctx")
                        l_ps = ps_l.tile([128, IB], F32, tag="l")
                        for idx, jt in enumerate(jts):
                            first, last = idx == 0, idx == len(jts) - 1
                            jcol = slice(jt * JT, (jt + 1) * JT)
                            st_ps = ps_mm.tile([128, IB], F32, tag="mm")
                            nc.tensor.matmul(st_ps, kT[h][:, jcol], qT[h][:, icol],
                                             start=True, stop=False)
                            # additive doc-causal mask: st += I.T @ M (M: 0/-1e9)
                            nc.tensor.matmul(st_ps, idn, mts[idx],
                                             start=False, stop=True)
                            pt = w3.tile([128, IB], BF, tag="exp")
                            nc.scalar.activation(pt, st_ps, Exp)
                            nc.tensor.matmul(ctx_ps, v_sb[:, jt, hcol], pt,
                                             start=first, stop=last)
                            nc.tensor.matmul(l_ps, ones, pt, start=first, stop=last)
                        # evacuate l/ctx banks fast (a slow reader here would
                        # stall the next accumulation group at the PE queue head)
                        l_sb = w2.tile([128, IB], F32, tag="lsb")
                        nc.vector.tensor_copy(l_sb, l_ps)
                        ctx_sb = w2.tile([128, IB], F32, tag="csb")
                        nc.vector.tensor_copy(ctx_sb, ctx_ps)
                        rec = w2.tile([128, IB], F32, tag="rec")
                        nc.vector.reciprocal(rec, l_sb)
                        nc.vector.tensor_mul(ctxT[h][:, icol], ctx_sb, rec)
                    # out rows for this block: out[s, e] = sum_h ctxT_h.T @ Wo_h
                    for eb in range(D // 512):
                        ecol = slice(eb * 512, (eb + 1) * 512)
                        for sl in range(IB // 128):
                            srow = slice(ib * IB + sl * 128,
                                         ib * IB + (sl + 1) * 128)
                            ops = ps_rot.tile([128, 512], F32, tag="rot")
                            for h in range(HPG):
                                nc.tensor.matmul(ops, ctxT[h][:, srow],
                                                 wo[:, h, ecol],
                                                 start=(h == 0), stop=(h == HPG - 1))
                            osb = w2.tile([128, 512], F32, tag="osb")
                            if (eb + sl) % 2 == 0:
                                nc.vector.tensor_copy(osb, ops)
                            else:
                                nc.scalar.copy(osb, ops)
                            nc.sync.dma_start(out=out_d[srow, ecol], in_=osb)

    orig = nc.to_json_bytes
    nc.to_json_bytes = lambda: _legalize_bir(orig())
    return nc


LAST_RUN = {}


def kernel(x, doc_ids, Wq, Wk, Wv, Wo, bo):
    import os
    from concourse.bass_utils import run_bass_kernel_spmd

    x = np.asarray(x, dtype=np.float32)
    doc_ids = np.asarray(doc_ids)
    Wq = np.asarray(Wq, dtype=np.float32)
    Wk = np.asarray(Wk, dtype=np.float32)
    Wv = np.asarray(Wv, dtype=np.float32)
    Wo = np.asarray(Wo, dtype=np.float32)
    bo = np.asarray(bo, dtype=np.float32)

    bf = ml_dtypes.bfloat16
    inv_scale = 1.0 / np.sqrt(HD)

    # doc structure -> shared (batch-unified) attention spans
    starts = [_doc_starts(doc_ids[b]) for b in range(B)]
    jts_per_ib = []
    for ib in range(N_IB):
        ja = min(int(starts[b][ib * IB]) for b in range(B))
        jts_per_ib.append(list(range(ja // JT, (ib + 1) * IB // JT)))

    # masks per batch: multiplicative {0,1}, doc-equality AND causal
    masks = []
    for b in range(B):
        drow = doc_ids[b]
        blocks = []
        for ib in range(N_IB):
            i0 = ib * IB
            for jt in jts_per_ib[ib]:
                j0 = jt * JT
                mj = drow[j0:j0 + JT, None] == drow[None, i0:i0 + IB]
                cz = (np.arange(j0, j0 + JT)[:, None]
                      <= np.arange(i0, i0 + IB)[None, :])
                allowed = mj & cz
                blocks.append(np.where(allowed, 0.0, -1e9).astype(bf))
        masks.append(np.stack(blocks) if blocks
                     else np.zeros((1, JT, IB), dtype=bf))

    rope = [_rope_tables(doc_ids[b]) for b in range(B)]
    rotm = np.zeros((HD, HD), dtype=bf)
    for t in range(HD // 2):
        rotm[2 * t + 1, 2 * t] = -1.0
        rotm[2 * t, 2 * t + 1] = 1.0

    xts = [np.ascontiguousarray(x[b].T).astype(bf) for b in range(B)]
    wq_s = (Wq * inv_scale).astype(bf)
    wk_s = Wk.astype(bf)
    wv_s = Wv.astype(bf)
    wo_s = Wo.astype(bf)

    in_maps = []
    for c in range(N_CORES):
        b, g = divmod(c, G)
        cols = slice(g * GC, (g + 1) * GC)
        in_maps.append({
            "xt": xts[b],
            "wq": np.ascontiguousarray(wq_s[:, cols]),
            "wk": np.ascontiguousarray(wk_s[:, cols]),
            "wv": np.ascontiguousarray(wv_s[:, cols]),
            "wo": np.ascontiguousarray(wo_s[cols, :]),
            "cosT": rope[b][0],
            "sinT": rope[b][1],
            "rotm": rotm,
            "idn": np.eye(128, dtype=bf),
            "masks": masks[b],
        })

    nc = _build_program(jts_per_ib)
    trace = bool(int(os.environ.get("KERNEL_TRACE", "0")))
    res = run_bass_kernel_spmd(nc, in_maps, core_ids=list(range(N_CORES)),
                               trace=trace)
    LAST_RUN["exec_time_ns"] = res.exec_time_ns
    LAST_RUN["mean_exec_time_ns"] = res.mean_exec_time_ns
    LAST_RUN["trace"] = res.instructions_and_trace

    out = np.empty((B, S, D), dtype=np.float32)
    for b in range(B):
        acc = np.zeros((S, D), dtype=np.float32)
        for g in range(G):
            acc += res.results[b * G + g]["part"]
        out[b] = acc + bo[None, :]
    return out


# revision 13
# speedup vs baseline: 1.2518x; 1.0638x over previous
"""Trainium2 Bass kernel: masked multi-head attention with doc-aware RoPE.

Problem shape: x[2, 2048, 2048], 16 heads x head_dim 128, doc-blockwise causal
mask with positions reset per document, out-proj with bias.

Sharding over 8 NeuronCores: core c = 4*b + g handles batch b (2) and head
group g (4 heads, i.e. 512 of the 2048 feature columns of Wq/Wk/Wv and 512
rows of Wo). Each core computes an out-proj partial [2048, 2048]; the host
sums the 4 partials per batch and adds the bias.

Device program (identical structure on all 8 cores; compiled per input batch
because the doc-boundary structure specializes the attention loop bounds):
  Phase 1: Q^T/K^T (layout [head_dim, S]) and V (layout [S, head_dim]) via
           matmuls against x^T; doc-aware RoPE applied to Q/K with a
           pair-rotation matmul (PE) + cos/sin elementwise combine (DVE).
  Phase 2: attention per (head, 512-query block): S^T tiles [128 keys, 512
           queries] on PE, multiplicative {0,1} doc-causal mask (host-built,
           streamed), exp on ACT (no max-subtraction: scores ~ N(0,1)),
           row-sums via all-ones matmul (lands replicated across partitions),
           P^T V accumulated into ctx^T in PSUM, normalized by reciprocal.
           Only key tiles in [doc_start(query block), query block end) are
           touched (doc-block sparsity).
  Phase 3: out partial[s, e] += ctx^T_h[t, s] @ Wo_h[t, e] accumulated over
           the 4 local heads in PSUM, streamed to DRAM.

All matmuls run in bf16 (1 cycle/row on the PE; fp32 would be 4x slower).
"""

import json

import numpy as np
import ml_dtypes

B, S, D, H = 2, 2048, 2048, 16
HD = D // H  # 128
ROPE_BASE = 10000.0
G = 4            # head groups (cores per batch)
HPG = H // G     # heads per group = 4
GC = HPG * HD    # feature cols per group = 512
N_CORES = 8
IB = 512         # query block size (phase 2)
N_IB = S // IB   # 4
JT = 128         # key tile size
BF = None        # set lazily (mybir.dt.bfloat16)
F32 = None

_DMA_OPS = {"DMACopy", "DMATranspose", "CCE", "DMAListExec"}


def _legalize_bir(raw: bytes) -> bytes:
    """This container's walrus accepts at most one sem-wait and one sem-update
    per instruction; Tile freely attaches several. Hoist extra waits onto
    NoOps inserted before the instruction (same engine stream -> the
    sequencer stalls there first, which is equivalent but legal) and extra
    updates onto NoOps after it (engine queues are FIFO, so the trailing nop's
    update fires after the instruction completes). DMA updates are left
    untouched: they ride the DGE descriptor, not the instruction."""
    d = json.loads(raw)
    n = 0
    for fn in d["functions"]:
        for blk in fn["blocks"]:
            out = []
            for inst in blk["instructions"]:
                si = inst.get("sync_info")
                if not si:
                    out.append(inst)
                    continue
                waits = list(si.get("on_wait") or [])
                upds = list(si.get("on_update") or [])
                dbg = {"debug": inst["debug"]} if "debug" in inst else {}
                while len(waits) > 1:
                    w = waits.pop(0)
                    n += 1
                    out.append({
                        "name": f"lglw-{n}", "opcode": "NoOp",
                        "engine": inst["engine"], "ins": [], "outs": [],
                        "sync_info": {"on_wait": [w], "on_update": []}, **dbg,
                    })
                si["on_wait"] = waits
                trailing = []
                if inst["opcode"] not in _DMA_OPS:
                    while len(upds) > 1:
                        u = upds.pop()
                        n += 1
                        trailing.append({
                            "name": f"lglu-{n}", "opcode": "NoOp",
                            "engine": inst["engine"], "ins": [], "outs": [],
                            "sync_info": {"on_wait": [], "on_update": [u]}, **dbg,
                        })
                    si["on_update"] = upds
                out.append(inst)
                out.extend(trailing)
            blk["instructions"] = out
    return json.dumps(d).encode()


def _doc_position_ids(doc_row: np.ndarray) -> np.ndarray:
    s = doc_row.shape[0]
    ar = np.arange(s)
    boundary = np.ones(s, dtype=bool)
    boundary[1:] = doc_row[1:] != doc_row[:-1]
    last = np.maximum.accumulate(np.where(boundary, ar, 0))
    return ar - last


def _doc_starts(doc_row: np.ndarray) -> np.ndarray:
    s = doc_row.shape[0]
    ar = np.arange(s)
    boundary = np.ones(s, dtype=bool)
    boundary[1:] = doc_row[1:] != doc_row[:-1]
    return np.maximum.accumulate(np.where(boundary, ar, 0))


def _rope_tables(doc_row: np.ndarray):
    pos = _doc_position_ids(doc_row).astype(np.float32)          # [S]
    inv_freq = 1.0 / (ROPE_BASE ** (np.arange(0, HD, 2, dtype=np.float32) / HD))
    freqs = pos[:, None] * inv_freq[None, :]                     # [S, 64]
    cos = np.repeat(np.cos(freqs), 2, axis=-1)                   # [S, 128]
    sin = np.repeat(np.sin(freqs), 2, axis=-1)
    to_bf = lambda a: np.ascontiguousarray(a.T).astype(ml_dtypes.bfloat16)
    return to_bf(cos), to_bf(sin)                                # [128, S]


def _build_program(jts_per_ib):
    import concourse.bass as bass
    import concourse.mybir as mybir
    from concourse.tile import TileContext

    BF = mybir.dt.bfloat16
    F32 = mybir.dt.float32
    Exp = mybir.ActivationFunctionType.Exp
    Ln = mybir.ActivationFunctionType.Ln
    NDT = D // 128  # 16 contraction tiles
    n_units = sum(len(j) for j in jts_per_ib)

    nc = bass.Bass("TRN2", num_devices=N_CORES)
    xt_d = nc.dram_tensor("xt", [D, S], BF, kind="ExternalInput")
    wq_d = nc.dram_tensor("wq", [D, GC], BF, kind="ExternalInput")
    wk_d = nc.dram_tensor("wk", [D, GC], BF, kind="ExternalInput")
    wv_d = nc.dram_tensor("wv", [D, GC], BF, kind="ExternalInput")
    wo_d = nc.dram_tensor("wo", [GC, D], BF, kind="ExternalInput")
    cos_d = nc.dram_tensor("cosT", [HD, S], BF, kind="ExternalInput")
    sin_d = nc.dram_tensor("sinT", [HD, S], BF, kind="ExternalInput")
    rot_d = nc.dram_tensor("rotm", [HD, HD], BF, kind="ExternalInput")
    idn_d = nc.dram_tensor("idn", [128, 128], BF, kind="ExternalInput")
    msk_d = nc.dram_tensor("masks", [max(n_units, 1), JT, IB], BF, kind="ExternalInput")
    out_d = nc.dram_tensor("part", [S, D], BF, kind="ExternalOutput")

    with TileContext(nc) as tc:
        with (
            tc.tile_pool(name="constA", bufs=1) as constA,
            tc.tile_pool(name="persist", bufs=1) as persist,
            tc.tile_pool(name="ps_mm", bufs=3, space="PSUM") as ps_mm,
            tc.tile_pool(name="ps_rot", bufs=2, space="PSUM") as ps_rot,
            tc.tile_pool(name="ps_ctx", bufs=2, space="PSUM") as ps_ctx,
            tc.tile_pool(name="ps_l", bufs=1, space="PSUM") as ps_l,
        ):
            # phase-2/3 constants (DMAs for these are emitted after phase 1)
            wo = constA.tile([128, HPG, D], BF, tag="wo")
            idn = constA.tile([128, 128], BF, tag="idn")
            ones = constA.tile([128, 128], BF, tag="ones")
            qT = [persist.tile([HD, S], BF, tag=f"q{h}", name=f"qT{h}") for h in range(HPG)]
            kT = [persist.tile([HD, S], BF, tag=f"k{h}", name=f"kT{h}") for h in range(HPG)]
            v_sb = persist.tile([128, NDT, GC], BF, tag="v")
            ctxT = [persist.tile([HD, S], BF, tag=f"c{h}", name=f"ctxT{h}") for h in range(HPG)]

            xt_view = xt_d.ap().rearrange("(dt p) s -> p dt s", p=128)

            # ============ Phase 1 (pools freed afterwards) ============
            with (
                tc.tile_pool(name="p1c", bufs=1) as p1c,
                tc.tile_pool(name="p1xt", bufs=34) as p1xt,
                tc.tile_pool(name="p1w", bufs=2) as p1w,
            ):
                wq = p1c.tile([128, NDT, GC], BF, tag="wq")
                wk = p1c.tile([128, NDT, GC], BF, tag="wk")
                wv = p1c.tile([128, NDT, GC], BF, tag="wv")
                cosT = p1c.tile([HD, S], BF, tag="cos")
                sinT = p1c.tile([HD, S], BF, tag="sin")
                rotm = p1c.tile([HD, HD], BF, tag="rot")
                # first x block + V weights first: compute starts ~15us in
                xt_first = []
                for dt in range(NDT):
                    t = p1xt.tile([128, IB], BF, tag="xt", name=f"xt0_{dt}")
                    eng = nc.sync if dt % 2 == 0 else nc.gpsimd
                    eng.dma_start(out=t, in_=xt_view[:, dt, 0:IB])
                    xt_first.append(t)
                for w_t, w_dram, eng in ((wv, wv_d, nc.sync), (wq, wq_d, nc.gpsimd),
                                         (wk, wk_d, nc.sync)):
                    eng.dma_start(
                        out=w_t,
                        in_=w_dram.ap().rearrange("(dt p) c -> p dt c", p=128))
                nc.gpsimd.dma_start(out=cosT, in_=cos_d[:, :])
                nc.gpsimd.dma_start(out=sinT, in_=sin_d[:, :])
                nc.sync.dma_start(out=rotm, in_=rot_d[:, :])
                nc.sync.dma_start(out=idn, in_=idn_d[:, :])
                nc.vector.memset(ones, 1.0)

                for sb in range(N_IB):
                    scol = slice(sb * IB, (sb + 1) * IB)
                    if sb == 0:
                        xt_t = xt_first
                    else:
                        xt_t = []
                        for dt in range(NDT):
                            t = p1xt.tile([128, IB], BF, tag="xt",
                                          name=f"xt{sb}_{dt}")
                            eng = nc.sync if dt % 2 == 0 else nc.gpsimd
                            eng.dma_start(out=t, in_=xt_view[:, dt, scol])
                            xt_t.append(t)

                    # V for the 4 s-tiles of this block: out[s 128, t 512]
                    for sl in range(IB // 128):
                        st = sb * 4 + sl
                        ps = ps_mm.tile([128, GC], F32, tag="mm")
                        for dt in range(NDT):
                            nc.tensor.matmul(
                                ps, xt_t[dt][:, sl * 128:(sl + 1) * 128],
                                wv[:, dt, :], start=(dt == 0), stop=(dt == NDT - 1))
                        nc.scalar.copy(v_sb[:, st, :], ps)

                    # Q^T / K^T per head with RoPE: out[t 128, s 512]
                    for w_t, dsts in ((wq, qT), (wk, kT)):
                        for h in range(HPG):
                            hcol = slice(h * HD, (h + 1) * HD)
                            ps = ps_mm.tile([128, IB], F32, tag="mm")
                            for dt in range(NDT):
                                nc.tensor.matmul(
                                    ps, w_t[:, dt, hcol], xt_t[dt],
                                    start=(dt == 0), stop=(dt == NDT - 1))
                            qtmp = p1w.tile([128, IB], BF, tag="qtmp")
                            nc.scalar.copy(qtmp, ps)
                            rps = ps_rot.tile([128, IB], F32, tag="rot")
                            nc.tensor.matmul(rps, rotm, qtmp, start=True, stop=True)
                            # RoPE combine: q*cos + rot(q)*sin
                            t1 = p1w.tile([128, IB], F32, tag="t1")
                            nc.vector.tensor_mul(t1, ps, cosT[:, scol])
                            t2 = p1w.tile([128, IB], F32, tag="t2")
                            nc.vector.tensor_mul(t2, rps, sinT[:, scol])
                            nc.vector.tensor_add(dsts[h][:, scol], t1, t2)

            # ============ Phase 2+3 interleaved per query block ============
            with (
                tc.tile_pool(name="mskp", bufs=n_units + 1) as mskp,
                tc.tile_pool(name="w2", bufs=2) as w2,
                tc.tile_pool(name="w3", bufs=3) as w3,
            ):
                nc.sync.dma_start(
                    out=wo, in_=wo_d.ap().rearrange("(h p) e -> p h e", p=128))
                unit_off = [0]
                for jts in jts_per_ib:
                    unit_off.append(unit_off[-1] + len(jts))
                all_mts = []
                for ib in range(N_IB):
                    mts = []
                    for idx, jt in enumerate(jts_per_ib[ib]):
                        mt = mskp.tile([JT, IB], BF, tag="msk", name=f"m{ib}_{jt}")
                        nc.sync.dma_start(out=mt, in_=msk_d[unit_off[ib] + idx])
                        mts.append(mt)
                    all_mts.append(mts)

                for ib in range(N_IB):
                    icol = slice(ib * IB, (ib + 1) * IB)
                    jts = jts_per_ib[ib]
                    mts = all_mts[ib]
                    for h in range(HPG):
                        hcol = slice(h * HD, (h + 1) * HD)
                        ctx_ps = ps_ctx.tile([128, IB], F32, tag="ctx")
                        l_ps = ps_l.tile([128, IB], F32, tag="l")
                        for idx, jt in enumerate(jts):
                            first, last = idx == 0, idx == len(jts) - 1
                            jcol = slice(jt * JT, (jt + 1) * JT)
                            st_ps = ps_mm.tile([128, IB], F32, tag="mm")
                            nc.tensor.matmul(st_ps, kT[h][:, jcol], qT[h][:, icol],
                                             start=True, stop=False)
                            # additive doc-causal mask: st += I.T @ M (M: 0/-1e9)
                            nc.tensor.matmul(st_ps, idn, mts[idx],
                                             start=False, stop=True)
                            pt = w3.tile([128, IB], BF, tag="exp")
                            nc.scalar.activation(pt, st_ps, Exp)
                            nc.tensor.matmul(ctx_ps, v_sb[:, jt, hcol], pt,
                                             start=first, stop=last)
                            nc.tensor.matmul(l_ps, ones, pt, start=first, stop=last)
                        # 1/l = exp(-ln l) on ACT (reads PSUM; frees the l
                        # bank after ~0.7us instead of a 3.3us DVE reciprocal)
                        lnl = w2.tile([128, IB], F32, tag="lnl")
                        nc.scalar.activation(lnl, l_ps, Ln)
                        rec = w2.tile([128, IB], F32, tag="rec")
                        nc.scalar.activation(rec, lnl, Exp, scale=-1.0)
                        nc.vector.tensor_mul(ctxT[h][:, icol], ctx_ps, rec)
                    # out rows for this block: out[s, e] = sum_h ctxT_h.T @ Wo_h
                    for eb in range(D // 512):
                        ecol = slice(eb * 512, (eb + 1) * 512)
                        for sl in range(IB // 128):
                            srow = slice(ib * IB + sl * 128,
                                         ib * IB + (sl + 1) * 128)
                            ops = ps_rot.tile([128, 512], F32, tag="rot")
                            for h in range(HPG):
                                nc.tensor.matmul(ops, ctxT[h][:, srow],
                                                 wo[:, h, ecol],
                                                 start=(h == 0), stop=(h == HPG - 1))
                            osb = w2.tile([128, 512], BF, tag="osb")
                            nc.vector.tensor_copy(osb, ops)
                            nc.sync.dma_start(out=out_d[srow, ecol], in_=osb)

    orig = nc.to_json_bytes
    nc.to_json_bytes = lambda: _legalize_bir(orig())
    return nc


LAST_RUN = {}


def kernel(x, doc_ids, Wq, Wk, Wv, Wo, bo):
    import os
    from concourse.bass_utils import run_bass_kernel_spmd

    x = np.asarray(x, dtype=np.float32)
    doc_ids = np.asarray(doc_ids)
    Wq = np.asarray(Wq, dtype=np.float32)
    Wk = np.asarray(Wk, dtype=np.float32)
    Wv = np.asarray(Wv, dtype=np.float32)
    Wo = np.asarray(Wo, dtype=np.float32)
    bo = np.asarray(bo, dtype=np.float32)

    bf = ml_dtypes.bfloat16
    inv_scale = 1.0 / np.sqrt(HD)

    # doc structure -> shared (batch-unified) attention spans
    starts = [_doc_starts(doc_ids[b]) for b in range(B)]
    jts_per_ib = []
    for ib in range(N_IB):
        ja = min(int(starts[b][ib * IB]) for b in range(B))
        jts_per_ib.append(list(range(ja // JT, (ib + 1) * IB // JT)))

    # masks per batch: multiplicative {0,1}, doc-equality AND causal
    masks = []
    for b in range(B):
        drow = doc_ids[b]
        blocks = []
        for ib in range(N_IB):
            i0 = ib * IB
            for jt in jts_per_ib[ib]:
                j0 = jt * JT
                mj = drow[j0:j0 + JT, None] == drow[None, i0:i0 + IB]
                cz = (np.arange(j0, j0 + JT)[:, None]
                      <= np.arange(i0, i0 + IB)[None, :])
                allowed = mj & cz
                blocks.append(np.where(allowed, 0.0, -1e9).astype(bf))
        masks.append(np.stack(blocks) if blocks
                     else np.zeros((1, JT, IB), dtype=bf))

    rope = [_rope_tables(doc_ids[b]) for b in range(B)]
    rotm = np.zeros((HD, HD), dtype=bf)
    for t in range(HD // 2):
        rotm[2 * t + 1, 2 * t] = -1.0
        rotm[2 * t, 2 * t + 1] = 1.0

    xts = [np.ascontiguousarray(x[b].T).astype(bf) for b in range(B)]
    wq_s = (Wq * inv_scale).astype(bf)
    wk_s = Wk.astype(bf)
    wv_s = Wv.astype(bf)
    wo_s = Wo.astype(bf)

    in_maps = []
    for c in range(N_CORES):
        b, g = divmod(c, G)
        cols = slice(g * GC, (g + 1) * GC)
        in_maps.append({
            "xt": xts[b],
            "wq": np.ascontiguousarray(wq_s[:, cols]),
            "wk": np.ascontiguousarray(wk_s[:, cols]),
            "wv": np.ascontiguousarray(wv_s[:, cols]),
            "wo": np.ascontiguousarray(wo_s[cols, :]),
            "cosT": rope[b][0],
            "sinT": rope[b][1],
            "rotm": rotm,
            "idn": np.eye(128, dtype=bf),
            "masks": masks[b],
        })

    nc = _build_program(jts_per_ib)
    trace = bool(int(os.environ.get("KERNEL_TRACE", "0")))
    res = run_bass_kernel_spmd(nc, in_maps, core_ids=list(range(N_CORES)),
                               trace=trace)
    LAST_RUN["exec_time_ns"] = res.exec_time_ns
    LAST_RUN["mean_exec_time_ns"] = res.mean_exec_time_ns
    LAST_RUN["trace"] = res.instructions_and_trace

    out = np.empty((B, S, D), dtype=np.float32)
    for b in range(B):
        acc = np.zeros((S, D), dtype=np.float32)
        for g in range(G):
            acc += res.results[b * G + g]["part"].astype(np.float32)
        out[b] = acc + bo[None, :]
    return out
